# revision 33
# baseline (speedup 1.0000x reference)
"""AdaptiveCombiner (kNN-LM style) Trainium2 kernel.

out[b,s,v] = lam * knn_prob + (1-lam) * softmax(neural_logit)
where knn_prob scatters per-token neighbor weights into the vocab dim.

Sharding: data parallel over the 2048 tokens -> 256 tokens/core x 8 cores.

v3 design (memory-roofline):
- logits stream in as fp8e4m3 (host cast), 8.2MB/core; output streams out
  as bf16 (host upcast), 16.4MB/core.
- H holds BOTH 128-token halves in SBUF as bf16 [128, 2, 32000]; the
  in-stream never waits on the out-stream (full duplex DMA).
- ONE ACT pass: exp reads each fp8 chunk in place (fp8 bytes land in the
  upper half of the chunk's own bf16 slot; the bf16 write pointer trails
  the fp8 read), row-sums fused via the ACT accumulator.
- The neighbor/MLP/weight chain runs per half, half 0 first, so gpsimd
  local_scatter strip production starts as early as possible (it is 66us
  of the budget and must hide under everything else).
- out: per half, DVE scales H by nscale in 8000-col blocks (64B-aligned
  so the 4x DVE mode engages), adds the knn strip per 2000 cols (2x),
  streams out on the sync ring. For half 1 (the tail after the serial
  exp), ACT takes two of the four scale blocks.
"""

import contextlib
import functools
import sys

import numpy as np

sys.path.insert(0, "/opt/trn_rl_repo")

import concourse.bass as bass
import concourse.bacc as bacc
import concourse.tile as tile
from concourse import mybir
from concourse.bass_utils import run_bass_kernel_spmd
from concourse.masks import make_identity

F32 = mybir.dt.float32
BF16 = mybir.dt.bfloat16
F8 = mybir.dt.float8e4
I32 = mybir.dt.int32
I16 = mybir.dt.int16
AF = mybir.ActivationFunctionType
OP = mybir.AluOpType

B, S, K, V = 2, 1024, 32, 32000
R = 6                      # log2(K)+1 rows of the distance mask
NOUT = 7                   # 2 + log2(K) network outputs
HID = 32
TEMP = 10.0
NCORES = 8
N = B * S                  # 2048 tokens
T = N // NCORES            # 256 tokens per core
HALVES = T // 128          # 2 partition tiles of 128 tokens
SC = 2000                  # local_scatter strip width (< 2048 HW limit)
NSC = V // SC              # 16 scatter strips per half
TSB = 8000                 # DVE/ACT scale block (64B-aligned in bf16)
NTSB = V // TSB            # 4 scale blocks per half

# in-chunk plans: small lead chunks on half 0 pull the first exp earlier.
CHUNKS0 = [2000, 2000, 4000, 4000, 4000, 8000, 8000]
CHUNKS1 = [8000, 8000, 8000, 8000]

# packed small-input layout (f32 element offsets within a [128, PACKW] DMA):
# one dense transfer instead of five descriptor-heavy strided ones.
PK_DIST = 0                 # [128, 2, 32] f32
PK_KM = 64                  # [128, 192] f32
PK_W1 = 256                 # [65, 32] f32 (rows 0-64)
PK_W2 = 288                 # [33, 7] f32 (rows 0-32)
PK_VALS = 296               # [128, 2, 32] i16 (32 f32 slots)
PACKW = 328

LAST = None                # BassKernelResults of the most recent run


def build_nc() -> bass.Bass:
    nc = bacc.Bacc()

    logits = nc.declare_dram_parameter("logits", [T, V], BF16, isOutput=False)
    pack = nc.declare_dram_parameter("pack", [128, PACKW], F32, isOutput=False)
    out = nc.declare_dram_parameter("out", [T, V], BF16, isOutput=True)

    with tile.TileContext(nc) as tc:
        with (
            tc.tile_pool(name="singles", bufs=1) as sg,
            tc.tile_pool(name="knn", bufs=10) as knnp,
            tc.tile_pool(name="psum", bufs=1, space="PSUM") as pp,
            tc.tile_pool(name="psum2", bufs=2, space="PSUM") as pp2,
        ):
            # ---------------- small inputs / constants ----------------
            # ONE dense packed DMA for every small input: the strided
            # dist/vals/weight transfers cost ~700 tiny descriptors and
            # delayed the first logits chunk by ~4us.
            pack_sb = sg.tile([128, PACKW], F32)
            nc.sync.dma_start(out=pack_sb[:], in_=pack[:])
            dist_sb = pack_sb[:, PK_DIST:PK_DIST + HALVES * K].rearrange(
                "p (h k) -> p h k", h=HALVES)
            kmask_sb = pack_sb[:, PK_KM:PK_KM + R * K]
            w1c_sb = pack_sb[0:2 * K + 1, PK_W1:PK_W1 + HID]
            w2c_sb = pack_sb[0:HID + 1, PK_W2:PK_W2 + NOUT]
            vals_i = pack_sb[:, PK_VALS:PK_VALS + K].bitcast(I16).rearrange(
                "p (h k) -> p h k", h=HALVES)
            # index grids on gpsimd: they gate the DVE prep chain.
            dji = sg.tile([128, K, K], I32)
            nc.gpsimd.iota(dji[:], pattern=[[1, K], [-1, K]],
                           channel_multiplier=0)
            cb_i = sg.tile([128, NSC], I32)
            nc.gpsimd.iota(cb_i[:], pattern=[[SC, NSC]], channel_multiplier=0)
            identity0 = sg.tile([128, 128], F32)
            make_identity(nc, identity0[:])

            # ---------------- in-stream: both halves up-front ----------
            # bf16 logits land directly in H; exp converts strictly in
            # place. (fp8 would halve the bytes but DMA cost here is
            # per-ELEMENT, so fp8 moves no faster and quantizes worse.)
            H = sg.tile([128, HALVES, V], BF16)
            CHUNKS = (CHUNKS0, CHUNKS1)
            sumacc0 = sg.tile([128, len(CHUNKS0)], F32)
            sumacc1 = sg.tile([128, len(CHUNKS1)], F32)
            sumacc = [sumacc0, sumacc1]
            for h in range(HALVES):
                base = 0
                for w in CHUNKS[h]:
                    nc.sync.dma_start(
                        out=H[:, h, base:base + w],
                        in_=logits[h * 128:(h + 1) * 128, base:base + w],
                    )
                    base += w

            def exp_chunk(h, c):
                base = sum(CHUNKS[h][:c])
                w = CHUNKS[h][c]
                nc.scalar.activation(
                    out=H[:, h, base:base + w],
                    in_=H[:, h, base:base + w],
                    func=AF.Exp,
                    accum_out=sumacc[h][:, c:c + 1],
                )

            exp_chunk(0, 0)
            exp_chunk(0, 1)

            # ---------------- prep: funnels, masks, memsets ------------
            identity = sg.tile([128, 128], F32)
            nc.vector.tensor_copy(out=identity[:], in_=identity0[:])
            dist_pe = sg.tile([128, HALVES, K], F32)
            nc.vector.tensor_copy(out=dist_pe[:], in_=dist_sb)
            w1c_pe = sg.tile([2 * K + 1, HID], F32)
            nc.vector.tensor_copy(out=w1c_pe[:], in_=w1c_sb)
            w2c_pe = sg.tile([HID + 1, NOUT], F32)
            nc.vector.tensor_copy(out=w2c_pe[:], in_=w2c_sb)
            lt16 = sg.tile([128, K, K], I16)
            nc.vector.tensor_scalar(out=lt16[:], in0=dji[:], scalar1=0,
                                    scalar2=None, op0=OP.is_gt)
            ut16 = sg.tile([128, K, K], I16)
            nc.vector.tensor_scalar(out=ut16[:], in0=dji[:], scalar1=0,
                                    scalar2=None, op0=OP.is_lt)
            cb16 = sg.tile([128, NSC], I16)
            nc.vector.tensor_copy(out=cb16[:], in_=cb_i[:])
            zeros16 = sg.tile([128, K], I16)
            nc.vector.memset(zeros16[:], 0)
            netin_sb = sg.tile([2 * K + 1, HALVES, 128], F32)
            nc.vector.memset(netin_sb[2 * K:2 * K + 1, :, :], 1.0)
            h_sb = sg.tile([HID + 1, HALVES, 128], F32)
            nc.vector.memset(h_sb[HID:HID + 1, :, :], 1.0)

            # per-half state tiles
            eq16 = sg.tile([128, HALVES, K, K], I16)
            scr16 = sg.tile([128, HALVES, K, K], I16)
            nd16 = sg.tile([128, HALVES, K], I16)
            counts = sg.tile([128, HALVES, K], F32)
            kp_exp = sg.tile([128, HALVES, NOUT], F32)
            s7 = sg.tile([128, HALVES], F32)
            invs7 = sg.tile([128, HALVES], F32)
            kp0n = sg.tile([128, HALVES], F32)
            lam = sg.tile([128, HALVES], F32)
            e = sg.tile([128, HALVES, R, K], F32)
            z = sg.tile([128, HALVES, R, K], F32)
            sumK = sg.tile([128, HALVES, R], F32)
            invsK = sg.tile([128, HALVES, R], F32)
            coef = sg.tile([128, HALVES, R], F32)
            wtmp = sg.tile([128, HALVES, R, K], F32)
            wl_bf = sg.tile([128, HALVES, K], BF16)
            idx16 = sg.tile([128, HALVES, NSC, K], I16)
            scrb = sg.tile([128, HALVES, K, K], BF16)
            nscale = sg.tile([128, HALVES], F32)
            kmr = kmask_sb.rearrange("p (r k) -> p r k", r=R)

            LOG2E = 1.4426950408889634
            LN2 = 0.6931471805599453

            def dve_tanh(h):
                """tanh on DVE via Pade(5,4) + clamp: keeps the wl critical
                chain off the in-order ACT queue (which is busy streaming
                the big exps). |err| <= 2.4e-3, saturating regime only."""
                u = sg.tile([HID, 128], F32, tag="tanh_u")
                nc.vector.tensor_copy(out=u[:], in_=h_pss[h][:])
                u2 = sg.tile([HID, 128], F32, tag="tanh_u2")
                nc.vector.tensor_tensor(out=u2[:], in0=u[:], in1=u[:],
                                        op=OP.mult)
                a = sg.tile([HID, 128], F32, tag="tanh_a")
                nc.vector.tensor_scalar(out=a[:], in0=u2[:], scalar1=105.0,
                                        scalar2=None, op0=OP.add)
                nc.vector.tensor_tensor(out=a[:], in0=a[:], in1=u2[:],
                                        op=OP.mult)
                nc.vector.tensor_scalar(out=a[:], in0=a[:], scalar1=945.0,
                                        scalar2=None, op0=OP.add)
                nc.vector.tensor_tensor(out=a[:], in0=a[:], in1=u[:],
                                        op=OP.mult)          # numerator*u
                d = sg.tile([HID, 128], F32, tag="tanh_d")
                nc.vector.tensor_scalar(out=d[:], in0=u2[:], scalar1=15.0,
                                        scalar2=420.0, op0=OP.mult,
                                        op1=OP.add)
                nc.vector.tensor_tensor(out=d[:], in0=d[:], in1=u2[:],
                                        op=OP.mult)
                nc.vector.tensor_scalar(out=d[:], in0=d[:], scalar1=945.0,
                                        scalar2=None, op0=OP.add)
                nc.vector.reciprocal(out=d[:], in_=d[:])
                nc.vector.tensor_tensor(out=a[:], in0=a[:], in1=d[:],
                                        op=OP.mult)
                nc.vector.tensor_scalar(out=h_sb[0:HID, h, :], in0=a[:],
                                        scalar1=1.0, scalar2=-1.0,
                                        op0=OP.min, op1=OP.max)

            def dve_exp_kp(h, kp_ps):
                """exp of the 7 router logits on DVE: x = k*ln2 + r,
                e^x = 2^k * poly(r). 2^k via the int32 exponent bit trick.
                Rel err ~2e-4; range |x| < 30 is ample."""
                x = sg.tile([128, NOUT], F32, tag="kpx")
                nc.vector.tensor_copy(out=x[:], in_=kp_ps[:])
                kf = sg.tile([128, NOUT], F32, tag="kpkf")
                ki = sg.tile([128, NOUT], I32, tag="kpki")
                # k = floor(x*log2e + 0.5)  (int32 convert truncates toward
                # zero; the +-0.5 pre-bias makes it round-to-nearest)
                sgn = sg.tile([128, NOUT], F32, tag="kpsgn")
                nc.vector.tensor_scalar(out=sgn[:], in0=x[:], scalar1=0.0,
                                        scalar2=0.5, op0=OP.is_ge,
                                        op1=OP.subtract)   # +0.5 / -0.5
                nc.vector.tensor_scalar(out=kf[:], in0=x[:], scalar1=LOG2E,
                                        scalar2=None, op0=OP.mult)
                nc.vector.tensor_tensor(out=ki[:], in0=kf[:], in1=sgn[:],
                                        op=OP.add)
                nc.vector.tensor_copy(out=kf[:], in_=ki[:])
                r = sg.tile([128, NOUT], F32, tag="kpr")
                nc.vector.scalar_tensor_tensor(out=r[:], in0=kf[:],
                                               scalar=-LN2, in1=x[:],
                                               op0=OP.mult, op1=OP.add)
                p = sg.tile([128, NOUT], F32, tag="kpp")
                nc.vector.tensor_scalar(out=p[:], in0=r[:],
                                        scalar1=1.0 / 3.0, scalar2=1.0,
                                        op0=OP.mult, op1=OP.add)
                nc.vector.tensor_tensor(out=p[:], in0=p[:], in1=r[:],
                                        op=OP.mult)
                nc.vector.tensor_scalar(out=p[:], in0=p[:], scalar1=0.5,
                                        scalar2=1.0, op0=OP.mult, op1=OP.add)
                nc.vector.tensor_tensor(out=p[:], in0=p[:], in1=r[:],
                                        op=OP.mult)
                nc.vector.tensor_scalar(out=p[:], in0=p[:], scalar1=1.0,
                                        scalar2=None, op0=OP.add)
                nc.vector.tensor_scalar(out=ki[:], in0=ki[:], scalar1=127,
                                        scalar2=1 << 23, op0=OP.add,
                                        op1=OP.mult)
                nc.vector.tensor_tensor(out=kp_exp[:, h], in0=p[:],
                                        in1=ki[:].bitcast(F32), op=OP.mult)
                nc.vector.tensor_reduce(out=s7[:, h:h + 1],
                                        in_=kp_exp[:, h],
                                        axis=mybir.AxisListType.X, op=OP.add)

            def prep_half(h, hp=False):
                """Neighbor dedup, counts, MLP, knn weights, scatter data
                for one 128-token half. The whole chain runs on DVE/PE
                (tanh and the 7-wide router exp are DVE polynomials), so
                strip production never waits on the in-order ACT queue."""
                def prio():
                    return tc.high_priority() if hp else contextlib.nullcontext()
                with prio():
                    prep_half_a(h)
                nc.scalar.activation(out=e[:, h], in_=z[:, h], func=AF.Exp)
                with prio():
                    dve_tanh(h)
                    kp_ps = pp2.tile([128, NOUT], F32, space="PSUM",
                                     tag=f"kpps{h}")
                    nc.tensor.matmul(out=kp_ps[:], lhsT=h_sb[:, h, :],
                                     rhs=w2c_pe[:], start=True, stop=True)
                    dve_exp_kp(h, kp_ps)
                    prep_half_b(h)

            def prep_half_a(h):
                # eq/dup/first-occurrence
                nc.vector.tensor_tensor(
                    out=eq16[:, h], in0=vals_i[:, h, :, None].to_broadcast(
                        [128, K, K]),
                    in1=vals_i[:, h, None, :].to_broadcast([128, K, K]),
                    op=OP.is_equal,
                )
                nc.vector.tensor_tensor(out=scr16[:, h], in0=eq16[:, h],
                                        in1=lt16[:], op=OP.mult)
                nc.gpsimd.tensor_tensor(out=scrb[:, h], in0=eq16[:, h],
                                        in1=ut16[:], op=OP.mult)
                dup = sg.tile([128, K], I16, tag=f"dup{h}")
                nc.vector.tensor_reduce(out=dup[:], in_=scr16[:, h],
                                        axis=mybir.AxisListType.X, op=OP.max)
                nc.vector.tensor_scalar(out=nd16[:, h], in0=dup[:], scalar1=0,
                                        scalar2=None, op0=OP.is_equal)
                nz = sg.tile([128, K], I16, tag=f"nz{h}")
                nc.vector.tensor_scalar(out=nz[:], in0=vals_i[:, h],
                                        scalar1=0, scalar2=None,
                                        op0=OP.not_equal)
                nf = sg.tile([128, K], I16, tag=f"nf{h}")
                nc.vector.tensor_tensor(out=nf[:], in0=nz[:], in1=nd16[:, h],
                                        op=OP.mult)
                nc.vector.tensor_tensor_scan(
                    out=counts[:, h], data0=nf[:], data1=zeros16[:],
                    initial=0.0, op0=OP.add, op1=OP.max,
                )
                # netin^T for this half: [dist; counts; ones]
                for row, src in ((0, dist_pe), (K, counts)):
                    tp = pp2.tile([K, 128], F32, space="PSUM")
                    nc.tensor.transpose(out=tp[:], in_=src[:, h, :],
                                        identity=identity[:])
                    nc.vector.tensor_copy(out=netin_sb[row:row + K, h, :],
                                          in_=tp[:])
                nc.tensor.matmul(out=h_pss[h][:], lhsT=w1c_pe[:],
                                 rhs=netin_sb[:, h, :], start=True, stop=True)
                # knn-softmax exp argument (independent of the MLP)
                nc.vector.tensor_tensor(
                    out=z[:, h],
                    in0=dist_sb[:, h, None, :].to_broadcast([128, R, K]),
                    in1=kmr[:, :, :], op=OP.mult,
                )

            def prep_half_b(h):
                nc.vector.reciprocal(out=invs7[:, h:h + 1], in_=s7[:, h:h + 1])
                nc.vector.tensor_tensor(out=kp0n[:, h:h + 1],
                                        in0=kp_exp[:, h, 0:1],
                                        in1=invs7[:, h:h + 1], op=OP.mult)
                nc.vector.tensor_scalar(out=lam[:, h:h + 1],
                                        in0=kp0n[:, h:h + 1], scalar1=-1.0,
                                        scalar2=1.0, op0=OP.mult, op1=OP.add)
                nc.vector.tensor_reduce(out=sumK[:, h], in_=e[:, h],
                                        axis=mybir.AxisListType.X, op=OP.add)
                nc.vector.reciprocal(out=invsK[:, h], in_=sumK[:, h])
                nc.vector.tensor_tensor(
                    out=coef[:, h], in0=kp_exp[:, h, 1:NOUT],
                    in1=invs7[:, h, None].to_broadcast([128, R]), op=OP.mult)
                nc.vector.tensor_tensor(out=coef[:, h], in0=coef[:, h],
                                        in1=invsK[:, h], op=OP.mult)
                nc.vector.tensor_tensor(
                    out=wtmp[:, h], in0=e[:, h],
                    in1=coef[:, h, :, None].to_broadcast([128, R, K]),
                    op=OP.mult,
                )
                w01 = sg.tile([128, K], F32, tag=f"w01{h}")
                w23 = sg.tile([128, K], F32, tag=f"w23{h}")
                w45 = sg.tile([128, K], F32, tag=f"w45{h}")
                nc.vector.tensor_tensor(out=w01[:], in0=wtmp[:, h, 0, :],
                                        in1=wtmp[:, h, 1, :], op=OP.add)
                nc.vector.tensor_tensor(out=w23[:], in0=wtmp[:, h, 2, :],
                                        in1=wtmp[:, h, 3, :], op=OP.add)
                nc.vector.tensor_tensor(out=w45[:], in0=wtmp[:, h, 4, :],
                                        in1=wtmp[:, h, 5, :], op=OP.add)
                nc.vector.tensor_tensor(out=w01[:], in0=w01[:], in1=w23[:],
                                        op=OP.add)
                nc.vector.tensor_tensor(out=w01[:], in0=w01[:], in1=w45[:],
                                        op=OP.add)
                wl = sg.tile([128, K], F32, tag=f"wl{h}")
                nc.vector.tensor_tensor(
                    out=wl[:], in0=w01[:],
                    in1=lam[:, h, None].to_broadcast([128, K]), op=OP.mult)
                # duplicate-weight fold into first occurrence
                wl_b = sg.tile([128, K], BF16, tag=f"wlb{h}")
                nc.vector.tensor_copy(out=wl_b[:], in_=wl[:])
                nc.vector.tensor_tensor(
                    out=scrb[:, h], in0=scrb[:, h],
                    in1=wl_b[:, None, :].to_broadcast([128, K, K]),
                    op=OP.mult,
                )
                wl_cmb = sg.tile([128, K], F32, tag=f"wlc{h}")
                nc.vector.tensor_reduce(out=wl_cmb[:], in_=scrb[:, h],
                                        axis=mybir.AxisListType.X, op=OP.add)
                nc.vector.tensor_tensor(out=wl_cmb[:], in0=wl_cmb[:],
                                        in1=wl[:], op=OP.add)
                nc.vector.tensor_copy(out=wl_bf[:, h], in_=wl_cmb[:])
                # strip-relative scatter indices for this half
                rel = sg.tile([128, NSC, K], I16, tag=f"rel{h}")
                nc.vector.tensor_tensor(
                    out=rel[:],
                    in0=vals_i[:, h, None, :].to_broadcast([128, NSC, K]),
                    in1=cb16[:, :, None].to_broadcast([128, NSC, K]),
                    op=OP.subtract,
                )
                vld = sg.tile([128, NSC, K], I16, tag=f"vld{h}")
                nc.vector.tensor_scalar(out=vld[:], in0=rel[:], scalar1=SC,
                                        scalar2=None, op0=OP.is_lt)
                nc.vector.tensor_tensor(
                    out=vld[:], in0=vld[:],
                    in1=nd16[:, h, None, :].to_broadcast([128, NSC, K]),
                    op=OP.mult,
                )
                nc.vector.tensor_tensor(out=rel[:], in0=rel[:], in1=vld[:],
                                        op=OP.mult)
                nc.vector.tensor_tensor(out=rel[:], in0=rel[:], in1=vld[:],
                                        op=OP.add)
                nc.vector.tensor_scalar(out=idx16[:, h], in0=rel[:],
                                        scalar1=-1, scalar2=None, op0=OP.add)
                # scatter strips for this half (gpsimd queue); local_scatter
                # needs a dense standalone tile (its lowering assumes
                # partition stride == num_elems), so strips live in a
                # rotating pool.
                for s in range(NSC):
                    knn = knnp.tile([128, SC], BF16)
                    nc.gpsimd.local_scatter(
                        out_ap=knn[:], data_ap=wl_bf[:, h, :],
                        idxs_ap=idx16[:, h, s, :],
                        channels=128, num_elems=SC, num_idxs=K,
                    )
                    knn_strips[h].append(knn[:])

            knn_strips = [[], []]
            h_ps0 = pp.tile([HID, 128], F32, space="PSUM")
            h_ps1 = pp.tile([HID, 128], F32, space="PSUM")
            h_pss = [h_ps0, h_ps1]
            prep_half(0, hp=True)
            exp_chunk(0, 2)
            prep_half(1)
            for c in range(3, len(CHUNKS0)):
                exp_chunk(0, c)
            for c in range(len(CHUNKS1)):
                exp_chunk(1, c)

            # ---------------- normalization + out stream ---------------
            def normalize(h):
                sumV = sg.tile([128, 1], F32, tag=f"sumV{h}")
                nc.vector.tensor_reduce(
                    out=sumV[:], in_=sumacc[h][:],
                    axis=mybir.AxisListType.X, op=OP.add,
                )
                invV = sg.tile([128, 1], F32, tag=f"invV{h}")
                nc.vector.reciprocal(out=invV[:], in_=sumV[:])
                nc.vector.tensor_tensor(
                    out=nscale[:, h:h + 1], in0=kp0n[:, h:h + 1],
                    in1=invV[:], op=OP.mult,
                )

            def ts_block(h, b, engine):
                bs = slice(b * TSB, (b + 1) * TSB)
                if engine == "act":
                    # Identity (not Copy): Copy with an AP scale hangs the
                    # ACT on bf16 in-place; Identity encodes bias as a
                    # const AP and works.
                    nc.scalar.activation(
                        out=H[:, h, bs], in_=H[:, h, bs], func=AF.Identity,
                        scale=nscale[:, h:h + 1],
                    )
                else:
                    nc.vector.tensor_scalar(
                        out=H[:, h, bs], in0=H[:, h, bs],
                        scalar1=nscale[:, h:h + 1], scalar2=None, op0=OP.mult,
                    )

            def tt_strips(h, s0, s1):
                for s in range(s0, s1):
                    ss = slice(s * SC, (s + 1) * SC)
                    nc.vector.tensor_tensor(
                        out=H[:, h, ss], in0=H[:, h, ss],
                        in1=knn_strips[h][s], op=OP.add,
                    )
                    nc.sync.dma_start(
                        out=out[h * 128:(h + 1) * 128, ss], in_=H[:, h, ss],
                    )

            SPB = TSB // SC          # strips per scale block
            normalize(0)
            for b in range(NTSB):
                ts_block(0, b, "dve")
                tt_strips(0, b * SPB, (b + 1) * SPB)
            normalize(1)
            # ACT (free after its last exp) scales blocks 2,3; DVE does
            # 0,1 and all the knn adds.
            ts_block(1, 2, "act")
            ts_block(1, 3, "act")
            ts_block(1, 0, "dve")
            tt_strips(1, 0, SPB)
            ts_block(1, 1, "dve")
            tt_strips(1, SPB, 4 * SPB)

    nc.compile()
    return nc


@functools.lru_cache(maxsize=1)
def get_nc() -> bass.Bass:
    return build_nc()


def make_in_maps(distances, values, neural_model_logit, W1, b1, W2, b2):
    import ml_dtypes

    distances = np.asarray(distances, dtype=np.float32).reshape(N, K)
    vals = np.asarray(values).reshape(N, K).astype(np.int16)
    logits = np.asarray(neural_model_logit, dtype=np.float32).reshape(N, V)
    logits8 = np.ascontiguousarray(logits.astype(ml_dtypes.bfloat16))
    w1c = np.concatenate(
        [np.asarray(W1, np.float32), np.asarray(b1, np.float32)[None]], 0)
    w2c = np.concatenate(
        [np.asarray(W2, np.float32), np.asarray(b2, np.float32)[None]], 0)
    p = 2 ** np.arange(R) - 1
    kmask = np.where(np.arange(K)[None, :] <= p[:, None], 1.0, 1000.0)
    kmn = (-kmask / TEMP).reshape(-1).astype(np.float32)
    in_maps = []
    for c in range(NCORES):
        sl = slice(c * T, (c + 1) * T)
        # packed small-input tensor: token t -> partition t%128, half t//128
        pk = np.zeros((128, PACKW), np.float32)
        d = distances[sl].reshape(HALVES, 128, K).transpose(1, 0, 2)
        pk[:, PK_DIST:PK_DIST + HALVES * K] = d.reshape(128, HALVES * K)
        pk[:, PK_KM:PK_KM + R * K] = kmn[None, :]
        pk[0:2 * K + 1, PK_W1:PK_W1 + HID] = w1c
        pk[0:HID + 1, PK_W2:PK_W2 + NOUT] = w2c
        v = vals[sl].reshape(HALVES, 128, K).transpose(1, 0, 2)
        pk[:, PK_VALS:PK_VALS + K] = np.ascontiguousarray(
            v.reshape(128, HALVES * K)).view(np.float32)
        in_maps.append(dict(logits=logits8[sl], pack=pk))
    return in_maps


def kernel(distances, values, neural_model_logit, W1, b1, W2, b2):
    global LAST
    in_maps = make_in_maps(distances, values, neural_model_logit,
                           W1, b1, W2, b2)
    nc = get_nc()
    LAST = run_bass_kernel_spmd(nc, in_maps, core_ids=list(range(NCORES)))
    outs = [LAST.results[i]["out"].astype(np.float32) for i in range(NCORES)]
    return np.concatenate(outs, 0).reshape(B, S, V)


# revision 34
# speedup vs baseline: 1.0459x; 1.0459x over previous
"""AdaptiveCombiner (kNN-LM style) Trainium2 kernel.

out[b,s,v] = lam * knn_prob + (1-lam) * softmax(neural_logit)
where knn_prob scatters per-token neighbor weights into the vocab dim.

Sharding: data parallel over the 2048 tokens -> 256 tokens/core x 8 cores.

v3 design (memory-roofline):
- logits stream in as fp8e4m3 (host cast), 8.2MB/core; output streams out
  as bf16 (host upcast), 16.4MB/core.
- H holds BOTH 128-token halves in SBUF as bf16 [128, 2, 32000]; the
  in-stream never waits on the out-stream (full duplex DMA).
- ONE ACT pass: exp reads each fp8 chunk in place (fp8 bytes land in the
  upper half of the chunk's own bf16 slot; the bf16 write pointer trails
  the fp8 read), row-sums fused via the ACT accumulator.
- The neighbor/MLP/weight chain runs per half, half 0 first, so gpsimd
  local_scatter strip production starts as early as possible (it is 66us
  of the budget and must hide under everything else).
- out: per half, DVE scales H by nscale in 8000-col blocks (64B-aligned
  so the 4x DVE mode engages), adds the knn strip per 2000 cols (2x),
  streams out on the sync ring. For half 1 (the tail after the serial
  exp), ACT takes two of the four scale blocks.
"""

import contextlib
import functools
import sys

import numpy as np

sys.path.insert(0, "/opt/trn_rl_repo")

import concourse.bass as bass
import concourse.bacc as bacc
import concourse.tile as tile
from concourse import mybir
from concourse.bass_utils import run_bass_kernel_spmd
from concourse.masks import make_identity

F32 = mybir.dt.float32
BF16 = mybir.dt.bfloat16
F8 = mybir.dt.float8e4
I32 = mybir.dt.int32
I16 = mybir.dt.int16
AF = mybir.ActivationFunctionType
OP = mybir.AluOpType

B, S, K, V = 2, 1024, 32, 32000
R = 6                      # log2(K)+1 rows of the distance mask
NOUT = 7                   # 2 + log2(K) network outputs
HID = 32
TEMP = 10.0
NCORES = 8
N = B * S                  # 2048 tokens
T = N // NCORES            # 256 tokens per core
HALVES = T // 128          # 2 partition tiles of 128 tokens
SC = 2000                  # local_scatter strip width (< 2048 HW limit)
NSC = V // SC              # 16 scatter strips per half
TSB = 8000                 # DVE/ACT scale block (64B-aligned in bf16)
NTSB = V // TSB            # 4 scale blocks per half

# in-chunk plans: small lead chunks on half 0 pull the first exp earlier.
CHUNKS0 = [2000, 2000, 4000, 4000, 4000, 8000, 8000]
CHUNKS1 = [8000, 8000, 8000, 8000]

# packed small-input layout (f32 element offsets within a [128, PACKW] DMA):
# one dense transfer instead of five descriptor-heavy strided ones.
PK_DIST = 0                 # [128, 2, 32] f32
PK_KM = 64                  # [128, 192] f32
PK_W1 = 256                 # [65, 32] f32 (rows 0-64)
PK_W2 = 288                 # [33, 7] f32 (rows 0-32)
PK_VALS = 296               # [128, 2, 32] i16 (32 f32 slots)
PACKW = 328

LAST = None                # BassKernelResults of the most recent run


def build_nc() -> bass.Bass:
    nc = bacc.Bacc()

    logits = nc.declare_dram_parameter("logits", [T, V], BF16, isOutput=False)
    pack = nc.declare_dram_parameter("pack", [128, PACKW], F32, isOutput=False)
    out = nc.declare_dram_parameter("out", [T, V], BF16, isOutput=True)

    with tile.TileContext(nc) as tc:
        with (
            tc.tile_pool(name="singles", bufs=1) as sg,
            tc.tile_pool(name="knn", bufs=11) as knnp,
            tc.tile_pool(name="psum", bufs=1, space="PSUM") as pp,
            tc.tile_pool(name="psum2", bufs=2, space="PSUM") as pp2,
        ):
            # ---------------- small inputs / constants ----------------
            # ONE dense packed DMA for every small input: the strided
            # dist/vals/weight transfers cost ~700 tiny descriptors and
            # delayed the first logits chunk by ~4us.
            pack_sb = sg.tile([128, PACKW], F32)
            nc.sync.dma_start(out=pack_sb[:], in_=pack[:])
            dist_sb = pack_sb[:, PK_DIST:PK_DIST + HALVES * K].rearrange(
                "p (h k) -> p h k", h=HALVES)
            kmask_sb = pack_sb[:, PK_KM:PK_KM + R * K]
            w1c_sb = pack_sb[0:2 * K + 1, PK_W1:PK_W1 + HID]
            w2c_sb = pack_sb[0:HID + 1, PK_W2:PK_W2 + NOUT]
            vals_i = pack_sb[:, PK_VALS:PK_VALS + K].bitcast(I16).rearrange(
                "p (h k) -> p h k", h=HALVES)
            # index grids on gpsimd: they gate the DVE prep chain.
            dji = sg.tile([128, K, K], I32)
            nc.gpsimd.iota(dji[:], pattern=[[1, K], [-1, K]],
                           channel_multiplier=0)
            cb_i = sg.tile([128, NSC], I32)
            nc.gpsimd.iota(cb_i[:], pattern=[[SC, NSC]], channel_multiplier=0)
            identity0 = sg.tile([128, 128], F32)
            make_identity(nc, identity0[:])

            # ---------------- in-stream: both halves up-front ----------
            # bf16 logits land directly in H; exp converts strictly in
            # place. (fp8 would halve the bytes but DMA cost here is
            # per-ELEMENT, so fp8 moves no faster and quantizes worse.)
            H = sg.tile([128, HALVES, V], BF16)
            CHUNKS = (CHUNKS0, CHUNKS1)
            sumacc0 = sg.tile([128, len(CHUNKS0)], F32)
            sumacc1 = sg.tile([128, len(CHUNKS1)], F32)
            sumacc = [sumacc0, sumacc1]
            for h in range(HALVES):
                base = 0
                for w in CHUNKS[h]:
                    nc.sync.dma_start(
                        out=H[:, h, base:base + w],
                        in_=logits[h * 128:(h + 1) * 128, base:base + w],
                    )
                    base += w

            def exp_chunk(h, c):
                base = sum(CHUNKS[h][:c])
                w = CHUNKS[h][c]
                nc.scalar.activation(
                    out=H[:, h, base:base + w],
                    in_=H[:, h, base:base + w],
                    func=AF.Exp,
                    accum_out=sumacc[h][:, c:c + 1],
                )

            exp_chunk(0, 0)
            exp_chunk(0, 1)

            # ---------------- prep: funnels, masks, memsets ------------
            identity = sg.tile([128, 128], F32)
            nc.vector.tensor_copy(out=identity[:], in_=identity0[:])
            dist_pe = sg.tile([128, HALVES, K], F32)
            nc.vector.tensor_copy(out=dist_pe[:], in_=dist_sb)
            w1c_pe = sg.tile([2 * K + 1, HID], F32)
            nc.vector.tensor_copy(out=w1c_pe[:], in_=w1c_sb)
            w2c_pe = sg.tile([HID + 1, NOUT], F32)
            nc.vector.tensor_copy(out=w2c_pe[:], in_=w2c_sb)
            lt16 = sg.tile([128, K, K], I16)
            nc.vector.tensor_scalar(out=lt16[:], in0=dji[:], scalar1=0,
                                    scalar2=None, op0=OP.is_gt)
            ut16 = sg.tile([128, K, K], I16)
            nc.vector.tensor_scalar(out=ut16[:], in0=dji[:], scalar1=0,
                                    scalar2=None, op0=OP.is_lt)
            cb16 = sg.tile([128, NSC], I16)
            nc.vector.tensor_copy(out=cb16[:], in_=cb_i[:])
            zeros16 = sg.tile([128, K], I16)
            nc.vector.memset(zeros16[:], 0)
            netin_sb = sg.tile([2 * K + 1, HALVES, 128], F32)
            nc.vector.memset(netin_sb[2 * K:2 * K + 1, :, :], 1.0)
            h_sb = sg.tile([HID + 1, HALVES, 128], F32)
            nc.vector.memset(h_sb[HID:HID + 1, :, :], 1.0)

            # per-half state tiles
            eq16 = sg.tile([128, HALVES, K, K], I16)
            scr16 = sg.tile([128, HALVES, K, K], I16)
            nd16 = sg.tile([128, HALVES, K], I16)
            counts = sg.tile([128, HALVES, K], F32)
            kp_exp = sg.tile([128, HALVES, NOUT], F32)
            s7 = sg.tile([128, HALVES], F32)
            invs7 = sg.tile([128, HALVES], F32)
            kp0n = sg.tile([128, HALVES], F32)
            lam = sg.tile([128, HALVES], F32)
            e = sg.tile([128, HALVES, R, K], F32)
            z = sg.tile([128, HALVES, R, K], F32)
            sumK = sg.tile([128, HALVES, R], F32)
            invsK = sg.tile([128, HALVES, R], F32)
            coef = sg.tile([128, HALVES, R], F32)
            wtmp = sg.tile([128, HALVES, R, K], F32)
            wl_bf = sg.tile([128, HALVES, K], BF16)
            idx16 = sg.tile([128, HALVES, NSC, K], I16)
            scrb = sg.tile([128, HALVES, K, K], BF16)
            nscale = sg.tile([128, HALVES], F32)
            kmr = kmask_sb.rearrange("p (r k) -> p r k", r=R)

            def prep_half(h, hp=False):
                """Neighbor dedup, counts, MLP, knn weights, scatter data
                for one 128-token half. ACT ops here slot between the
                streaming exp chunks in ACT program order; the DVE/PE ops
                of half 0 get scheduler priority (they gate the 66us of
                gpsimd strip production)."""
                def prio():
                    return tc.high_priority() if hp else contextlib.nullcontext()
                with prio():
                    prep_half_a(h)
                nc.scalar.activation(out=h_sb0s[h][:], in_=h_pss[h][:],
                                     func=AF.Tanh)
                nc.scalar.activation(out=e[:, h], in_=z[:, h], func=AF.Exp)
                with prio():
                    nc.vector.tensor_copy(out=h_sb[0:HID, h, :],
                                          in_=h_sb0s[h][:])
                    kp_ps = pp2.tile([128, NOUT], F32, space="PSUM",
                                     tag=f"kpps{h}")
                    nc.tensor.matmul(out=kp_ps[:], lhsT=h_sb[:, h, :],
                                     rhs=w2c_pe[:], start=True, stop=True)
                nc.scalar.activation(out=kp_exp[:, h], in_=kp_ps[:],
                                     func=AF.Exp, accum_out=s7[:, h:h + 1])
                with prio():
                    prep_half_b(h)

            def prep_half_a(h):
                # eq/dup/first-occurrence
                nc.vector.tensor_tensor(
                    out=eq16[:, h], in0=vals_i[:, h, :, None].to_broadcast(
                        [128, K, K]),
                    in1=vals_i[:, h, None, :].to_broadcast([128, K, K]),
                    op=OP.is_equal,
                )
                nc.vector.tensor_tensor(out=scr16[:, h], in0=eq16[:, h],
                                        in1=lt16[:], op=OP.mult)
                nc.vector.tensor_tensor(out=scrb[:, h], in0=eq16[:, h],
                                        in1=ut16[:], op=OP.mult)
                dup = sg.tile([128, K], I16, tag=f"dup{h}")
                nc.vector.tensor_reduce(out=dup[:], in_=scr16[:, h],
                                        axis=mybir.AxisListType.X, op=OP.max)
                nc.vector.tensor_scalar(out=nd16[:, h], in0=dup[:], scalar1=0,
                                        scalar2=None, op0=OP.is_equal)
                nz = sg.tile([128, K], I16, tag=f"nz{h}")
                nc.vector.tensor_scalar(out=nz[:], in0=vals_i[:, h],
                                        scalar1=0, scalar2=None,
                                        op0=OP.not_equal)
                nf = sg.tile([128, K], I16, tag=f"nf{h}")
                nc.vector.tensor_tensor(out=nf[:], in0=nz[:], in1=nd16[:, h],
                                        op=OP.mult)
                nc.vector.tensor_tensor_scan(
                    out=counts[:, h], data0=nf[:], data1=zeros16[:],
                    initial=0.0, op0=OP.add, op1=OP.max,
                )
                # netin^T for this half: [dist; counts; ones]
                for row, src in ((0, dist_pe), (K, counts)):
                    tp = pp2.tile([K, 128], F32, space="PSUM")
                    nc.tensor.transpose(out=tp[:], in_=src[:, h, :],
                                        identity=identity[:])
                    nc.vector.tensor_copy(out=netin_sb[row:row + K, h, :],
                                          in_=tp[:])
                nc.tensor.matmul(out=h_pss[h][:], lhsT=w1c_pe[:],
                                 rhs=netin_sb[:, h, :], start=True, stop=True)
                # knn-softmax exp argument (independent of the MLP)
                nc.vector.tensor_tensor(
                    out=z[:, h],
                    in0=dist_sb[:, h, None, :].to_broadcast([128, R, K]),
                    in1=kmr[:, :, :], op=OP.mult,
                )

            def prep_half_b(h):
                nc.vector.reciprocal(out=invs7[:, h:h + 1], in_=s7[:, h:h + 1])
                nc.vector.tensor_tensor(out=kp0n[:, h:h + 1],
                                        in0=kp_exp[:, h, 0:1],
                                        in1=invs7[:, h:h + 1], op=OP.mult)
                nc.vector.tensor_scalar(out=lam[:, h:h + 1],
                                        in0=kp0n[:, h:h + 1], scalar1=-1.0,
                                        scalar2=1.0, op0=OP.mult, op1=OP.add)
                nc.vector.tensor_reduce(out=sumK[:, h], in_=e[:, h],
                                        axis=mybir.AxisListType.X, op=OP.add)
                nc.vector.reciprocal(out=invsK[:, h], in_=sumK[:, h])
                nc.vector.tensor_tensor(
                    out=coef[:, h], in0=kp_exp[:, h, 1:NOUT],
                    in1=invs7[:, h, None].to_broadcast([128, R]), op=OP.mult)
                nc.vector.tensor_tensor(out=coef[:, h], in0=coef[:, h],
                                        in1=invsK[:, h], op=OP.mult)
                nc.vector.tensor_tensor(
                    out=wtmp[:, h], in0=e[:, h],
                    in1=coef[:, h, :, None].to_broadcast([128, R, K]),
                    op=OP.mult,
                )
                w01 = sg.tile([128, K], F32, tag=f"w01{h}")
                w23 = sg.tile([128, K], F32, tag=f"w23{h}")
                w45 = sg.tile([128, K], F32, tag=f"w45{h}")
                nc.vector.tensor_tensor(out=w01[:], in0=wtmp[:, h, 0, :],
                                        in1=wtmp[:, h, 1, :], op=OP.add)
                nc.vector.tensor_tensor(out=w23[:], in0=wtmp[:, h, 2, :],
                                        in1=wtmp[:, h, 3, :], op=OP.add)
                nc.vector.tensor_tensor(out=w45[:], in0=wtmp[:, h, 4, :],
                                        in1=wtmp[:, h, 5, :], op=OP.add)
                nc.vector.tensor_tensor(out=w01[:], in0=w01[:], in1=w23[:],
                                        op=OP.add)
                nc.vector.tensor_tensor(out=w01[:], in0=w01[:], in1=w45[:],
                                        op=OP.add)
                wl = sg.tile([128, K], F32, tag=f"wl{h}")
                nc.vector.tensor_tensor(
                    out=wl[:], in0=w01[:],
                    in1=lam[:, h, None].to_broadcast([128, K]), op=OP.mult)
                # duplicate-weight fold into first occurrence
                wl_b = sg.tile([128, K], BF16, tag=f"wlb{h}")
                nc.vector.tensor_copy(out=wl_b[:], in_=wl[:])
                nc.vector.tensor_tensor(
                    out=scrb[:, h], in0=scrb[:, h],
                    in1=wl_b[:, None, :].to_broadcast([128, K, K]),
                    op=OP.mult,
                )
                wl_cmb = sg.tile([128, K], F32, tag=f"wlc{h}")
                nc.vector.tensor_reduce(out=wl_cmb[:], in_=scrb[:, h],
                                        axis=mybir.AxisListType.X, op=OP.add)
                nc.vector.tensor_tensor(out=wl_cmb[:], in0=wl_cmb[:],
                                        in1=wl[:], op=OP.add)
                nc.vector.tensor_copy(out=wl_bf[:, h], in_=wl_cmb[:])
                # strip-relative scatter indices for this half
                rel = sg.tile([128, NSC, K], I16, tag=f"rel{h}")
                nc.vector.tensor_tensor(
                    out=rel[:],
                    in0=vals_i[:, h, None, :].to_broadcast([128, NSC, K]),
                    in1=cb16[:, :, None].to_broadcast([128, NSC, K]),
                    op=OP.subtract,
                )
                vld = sg.tile([128, NSC, K], I16, tag=f"vld{h}")
                nc.vector.tensor_scalar(out=vld[:], in0=rel[:], scalar1=SC,
                                        scalar2=None, op0=OP.is_lt)
                nc.vector.tensor_tensor(
                    out=vld[:], in0=vld[:],
                    in1=nd16[:, h, None, :].to_broadcast([128, NSC, K]),
                    op=OP.mult,
                )
                nc.vector.tensor_tensor(out=rel[:], in0=rel[:], in1=vld[:],
                                        op=OP.mult)
                nc.vector.tensor_tensor(out=rel[:], in0=rel[:], in1=vld[:],
                                        op=OP.add)
                nc.vector.tensor_scalar(out=idx16[:, h], in0=rel[:],
                                        scalar1=-1, scalar2=None, op0=OP.add)
                # scatter strips for this half (gpsimd queue); local_scatter
                # needs a dense standalone tile (its lowering assumes
                # partition stride == num_elems), so strips live in a
                # rotating pool.
                for s in range(NSC):
                    knn = knnp.tile([128, SC], BF16)
                    nc.gpsimd.local_scatter(
                        out_ap=knn[:], data_ap=wl_bf[:, h, :],
                        idxs_ap=idx16[:, h, s, :],
                        channels=128, num_elems=SC, num_idxs=K,
                    )
                    knn_strips[h].append(knn[:])

            knn_strips = [[], []]
            h_ps0 = pp.tile([HID, 128], F32, space="PSUM")
            h_ps1 = pp.tile([HID, 128], F32, space="PSUM")
            h_sb00 = sg.tile([HID, 128], F32)
            h_sb01 = sg.tile([HID, 128], F32)
            h_pss = [h_ps0, h_ps1]
            h_sb0s = [h_sb00, h_sb01]
            prep_half(0, hp=True)
            exp_chunk(0, 2)
            prep_half(1)
            for c in range(3, len(CHUNKS0)):
                exp_chunk(0, c)
            for c in range(len(CHUNKS1)):
                exp_chunk(1, c)

            # ---------------- normalization + out stream ---------------
            def normalize(h):
                sumV = sg.tile([128, 1], F32, tag=f"sumV{h}")
                nc.vector.tensor_reduce(
                    out=sumV[:], in_=sumacc[h][:],
                    axis=mybir.AxisListType.X, op=OP.add,
                )
                invV = sg.tile([128, 1], F32, tag=f"invV{h}")
                nc.vector.reciprocal(out=invV[:], in_=sumV[:])
                nc.vector.tensor_tensor(
                    out=nscale[:, h:h + 1], in0=kp0n[:, h:h + 1],
                    in1=invV[:], op=OP.mult,
                )

            def ts_block(h, b, engine):
                bs = slice(b * TSB, (b + 1) * TSB)
                if engine == "act":
                    # Identity (not Copy): Copy with an AP scale hangs the
                    # ACT on bf16 in-place; Identity encodes bias as a
                    # const AP and works.
                    nc.scalar.activation(
                        out=H[:, h, bs], in_=H[:, h, bs], func=AF.Identity,
                        scale=nscale[:, h:h + 1],
                    )
                else:
                    nc.vector.tensor_scalar(
                        out=H[:, h, bs], in0=H[:, h, bs],
                        scalar1=nscale[:, h:h + 1], scalar2=None, op0=OP.mult,
                    )

            def tt_strips(h, s0, s1):
                for s in range(s0, s1):
                    ss = slice(s * SC, (s + 1) * SC)
                    nc.vector.tensor_tensor(
                        out=H[:, h, ss], in0=H[:, h, ss],
                        in1=knn_strips[h][s], op=OP.add,
                    )
                    nc.sync.dma_start(
                        out=out[h * 128:(h + 1) * 128, ss], in_=H[:, h, ss],
                    )

            SPB = TSB // SC          # strips per scale block
            normalize(0)
            for b in range(NTSB):
                ts_block(0, b, "dve")
                tt_strips(0, b * SPB, (b + 1) * SPB)
            normalize(1)
            # ACT (free after its last exp) scales blocks 2,3; DVE does
            # 0,1 and all the knn adds.
            ts_block(1, 2, "act")
            ts_block(1, 3, "act")
            ts_block(1, 0, "dve")
            tt_strips(1, 0, SPB)
            ts_block(1, 1, "dve")
            tt_strips(1, SPB, 4 * SPB)

    nc.compile()
    return nc


@functools.lru_cache(maxsize=1)
def get_nc() -> bass.Bass:
    return build_nc()


def make_in_maps(distances, values, neural_model_logit, W1, b1, W2, b2):
    import ml_dtypes

    distances = np.asarray(distances, dtype=np.float32).reshape(N, K)
    vals = np.asarray(values).reshape(N, K).astype(np.int16)
    logits = np.asarray(neural_model_logit, dtype=np.float32).reshape(N, V)
    logits8 = np.ascontiguousarray(logits.astype(ml_dtypes.bfloat16))
    w1c = np.concatenate(
        [np.asarray(W1, np.float32), np.asarray(b1, np.float32)[None]], 0)
    w2c = np.concatenate(
        [np.asarray(W2, np.float32), np.asarray(b2, np.float32)[None]], 0)
    p = 2 ** np.arange(R) - 1
    kmask = np.where(np.arange(K)[None, :] <= p[:, None], 1.0, 1000.0)
    kmn = (-kmask / TEMP).reshape(-1).astype(np.float32)
    in_maps = []
    for c in range(NCORES):
        sl = slice(c * T, (c + 1) * T)
        # packed small-input tensor: token t -> partition t%128, half t//128
        pk = np.zeros((128, PACKW), np.float32)
        d = distances[sl].reshape(HALVES, 128, K).transpose(1, 0, 2)
        pk[:, PK_DIST:PK_DIST + HALVES * K] = d.reshape(128, HALVES * K)
        pk[:, PK_KM:PK_KM + R * K] = kmn[None, :]
        pk[0:2 * K + 1, PK_W1:PK_W1 + HID] = w1c
        pk[0:HID + 1, PK_W2:PK_W2 + NOUT] = w2c
        v = vals[sl].reshape(HALVES, 128, K).transpose(1, 0, 2)
        pk[:, PK_VALS:PK_VALS + K] = np.ascontiguousarray(
            v.reshape(128, HALVES * K)).view(np.float32)
        in_maps.append(dict(logits=logits8[sl], pack=pk))
    return in_maps


def kernel(distances, values, neural_model_logit, W1, b1, W2, b2):
    global LAST
    in_maps = make_in_maps(distances, values, neural_model_logit,
                           W1, b1, W2, b2)
    nc = get_nc()
    LAST = run_bass_kernel_spmd(nc, in_maps, core_ids=list(range(NCORES)))
    outs = [LAST.results[i]["out"].astype(np.float32) for i in range(NCORES)]
    return np.concatenate(outs, 0).reshape(B, S, V)
